# revision 16
# baseline (speedup 1.0000x reference)
"""Trainium2 Bass kernel for nn_Net_31722628448620.

2-layer bidirectional LSTM (B=64, T=1024, E=32, H=100, V=21) + two linear
heads (f: [B*T,20], g on adjacent-pair concat: [B*(T-1),400]).

Sharding: data-parallel over batch, 8 sequences per NeuronCore, weights
replicated. All real FLOPs run on device; the host only re-packs layouts.

Key device-side structure (per core, BC=8 sequences):
  - Layer 0 folds the embedding + input projection into the recurrent
    matmul: x_t = emb[token] lives in a V=21-dim space, so with
    Wihp = Wih @ emb.T the per-step input term is Wihp @ onehot(token).
    Contraction K = H(100) + V(21) + 1(bias) = 122 <= 128, so each gate
    is ONE matmul per step against an augmented state tile
    [h_{t-1}; onehot(token_t); 1].
  - The augmented state tile S doubles as the layer-output storage:
    step t's DVE h-write lands exactly where step t+1's matmul reads.
  - Layer 1's input transform Zx1 = Wih1 @ concat(hf0, hb0) is a big
    streaming matmul between the layer phases; during the layer-1
    recurrence it is injected into PSUM with an identity matmul
    (accumulating on top of the 4 gate matmuls), so the per-step
    elementwise work stays at 3 ACT + 3 DVE instructions per direction.
  - The pairwise concat for g is free: g's four K-chunks are just
    time-shifted slices of the layer-1 state storage.

Outputs are produced transposed ([20, T*8] / [400, (T-1)*8] per core);
the host assembles/transposes into the reference layout.
"""

import sys
import os

for _p in ("/opt/trn_rl_repo", "/root/.axon_site/_ro/trn_rl_repo"):
    if os.path.isdir(_p) and _p not in sys.path:
        sys.path.append(_p)

import numpy as np
import ml_dtypes

import concourse.bass as bass
import concourse.tile as tile
from concourse import mybir, bacc
from concourse.bass_utils import run_bass_kernel_spmd

BF16 = mybir.dt.bfloat16
F32 = mybir.dt.float32
AF = mybir.ActivationFunctionType

B, T_FULL, E, H, V = 64, 1024, 32, 100, 21
LOUT = 20
NCORES = 8
BC = B // NCORES  # sequences per core (8)

# gate order used on-device: [o, f, i, g]; torch weight row order is i,f,g,o
_GATE_ROWS = [slice(3 * H, 4 * H), slice(H, 2 * H), slice(0, H), slice(2 * H, 3 * H)]


def _build_nc(T: int, dump: bool = False) -> bass.Bass:
    """Build the full Bass program for one core (SPMD across 8)."""
    TB = T * BC            # time-major col count of one direction's states
    NBLK = (T + 1) * BC    # + one extra block for the initial state

    nc = bacc.Bacc("TRN2", target_bir_lowering=False, debug=False,
                   num_devices=NCORES)
    dS0 = dS1 = dZx1 = None
    if dump:
        dS0 = nc.dram_tensor("dS0", [128, 2, NBLK], BF16, kind="ExternalOutput")
        dS1 = nc.dram_tensor("dS1", [128, 2, NBLK], BF16, kind="ExternalOutput")
        dZx1 = nc.dram_tensor("dZx1", [H, 2, 4, TB], BF16, kind="ExternalOutput")

    # ---- I/O ----
    # oh carries V onehot rows + a constant-1.0 row (compute engines cannot
    # address a 1-partition slice at partition 121, so the bias row comes
    # in via DMA with the onehots)
    oh = nc.dram_tensor("oh", [28, TB], BF16, kind="ExternalInput")
    ones1 = nc.dram_tensor("ones1", [28, 2 * NBLK], BF16, kind="ExternalInput")
    w0 = nc.dram_tensor("w0", [128, 8, H], BF16, kind="ExternalInput")
    w1 = nc.dram_tensor("w1", [128, 8, H], BF16, kind="ExternalInput")
    w1x = nc.dram_tensor("w1x", [H, 16, H], BF16, kind="ExternalInput")
    ident = nc.dram_tensor("ident", [H, H], BF16, kind="ExternalInput")
    fw = nc.dram_tensor("fw", [H, 2, LOUT], BF16, kind="ExternalInput")
    fbias = nc.dram_tensor("fbias", [LOUT, 1], F32, kind="ExternalInput")
    gw = nc.dram_tensor("gw", [H, 16, H], BF16, kind="ExternalInput")
    gbias = nc.dram_tensor("gbias", [H, 4], F32, kind="ExternalInput")
    h0sb = nc.dram_tensor("h0sb", [H, 4, BC], BF16, kind="ExternalInput")
    c0s = nc.dram_tensor("c0s", [H, 4, BC], F32, kind="ExternalInput")

    fo = nc.dram_tensor("fo", [LOUT, TB], F32, kind="ExternalOutput")
    go = nc.dram_tensor("go", [4 * H, (T - 1) * BC], F32, kind="ExternalOutput")

    with tile.TileContext(nc) as tc:
        with (
            tc.tile_pool(name="singles", bufs=1) as singles,
            tc.tile_pool(name="stateA", bufs=1) as stateA,
            tc.tile_pool(name="stateB", bufs=1) as stateB,
        ):
            # ---- load weights / constants ----
            sw0 = singles.tile([128, 8, H], BF16)
            nc.gpsimd.dma_start(out=sw0, in_=w0[:, :, :])
            sw1 = singles.tile([128, 8, H], BF16)
            nc.gpsimd.dma_start(out=sw1, in_=w1[:, :, :])
            sw1x = singles.tile([H, 16, H], BF16)
            nc.gpsimd.dma_start(out=sw1x, in_=w1x[:, :, :])
            sid = singles.tile([H, H], BF16)
            nc.gpsimd.dma_start(out=sid, in_=ident[:, :])
            sfw = singles.tile([H, 2, LOUT], BF16)
            nc.gpsimd.dma_start(out=sfw, in_=fw[:, :, :])
            sfb = singles.tile([LOUT, 1], F32)
            nc.gpsimd.dma_start(out=sfb, in_=fbias[:, :])
            sgw = singles.tile([H, 16, H], BF16)
            nc.gpsimd.dma_start(out=sgw, in_=gw[:, :, :])
            sgb = singles.tile([H, 4], F32)
            nc.gpsimd.dma_start(out=sgb, in_=gbias[:, :])

            # ---- state storage ----
            # S0: [128 partitions, dir, (T+1)*BC]
            #   rows 0:100   h states of layer-0
            #   rows 100:121 onehot(token) (per direction, shifted)
            #   row 121      constant 1.0 (bias row for the aug matmul)
            S0 = stateA.tile([128, 2, NBLK], BF16)
            # forward: block t = [h_{t-1}; onehot(tok_t); 1]; h_t -> block t+1
            nc.gpsimd.dma_start(out=S0[H:128, 0, 0:TB], in_=oh[:, :])
            # backward: block j = [h_b(j); onehot(tok_{j-1}); 1] (j=1..T);
            # h_b(t) -> block t; initial state at block T
            nc.gpsimd.dma_start(out=S0[H:128, 1, BC:BC + TB], in_=oh[:, :])
            nc.gpsimd.dma_start(out=S0[0:H, 0, 0:BC], in_=h0sb[:, 0, :])
            nc.gpsimd.dma_start(out=S0[0:H, 1, T * BC:(T + 1) * BC],
                                in_=h0sb[:, 1, :])

            # S1: layer-1 h states; row 100 = 1.0 (bias row)
            S1 = stateB.tile([128, 2, NBLK], BF16)
            nc.gpsimd.dma_start(out=S1[H:128, :, :], in_=ones1[:, :])
            nc.gpsimd.dma_start(out=S1[0:H, 0, 0:BC], in_=h0sb[:, 2, :])
            nc.gpsimd.dma_start(out=S1[0:H, 1, T * BC:(T + 1) * BC],
                                in_=h0sb[:, 3, :])

            # persistent [c | ghat] per (layer, dir)
            cg0 = singles.tile([H, 2, 2, BC], F32)
            nc.gpsimd.dma_start(out=cg0[:, 0, 0, :], in_=c0s[:, 0, :])
            nc.gpsimd.dma_start(out=cg0[:, 1, 0, :], in_=c0s[:, 1, :])
            cg1 = singles.tile([H, 2, 2, BC], F32)
            nc.gpsimd.dma_start(out=cg1[:, 0, 0, :], in_=c0s[:, 2, :])
            nc.gpsimd.dma_start(out=cg1[:, 1, 0, :], in_=c0s[:, 3, :])

            # ---- recurrence helper ----
            def lstm_steps(S, cg, wtile, krows, zx):
                """Emit one bidirectional layer's T steps."""
                with (
                    tc.tile_pool(name="zp0", bufs=4, space="PSUM") as zp0,
                    tc.tile_pool(name="zp1", bufs=4, space="PSUM") as zp1,
                    tc.tile_pool(name="ew0", bufs=3) as ew0,
                    tc.tile_pool(name="ew1", bufs=3) as ew1,
                ):
                    zpools = (zp0, zp1)
                    epools = (ew0, ew1)
                    for t in range(T):
                        for d in (0, 1):
                            tseq = t if d == 0 else T - 1 - t
                            rd = tseq if d == 0 else tseq + 1
                            wr = tseq + 1 if d == 0 else tseq
                            Z = zpools[d].tile([H, 4, BC], F32, tag="z")
                            rhs = S[0:krows, d, rd * BC:(rd + 1) * BC]
                            if zx is not None:
                                # inject Zx1 first (identity matmul), gates
                                # then accumulate on top of it
                                nc.tensor.matmul(
                                    Z[:, :, :], sid,
                                    zx[:, d, :, tseq * BC:(tseq + 1) * BC],
                                    start=True, stop=False,
                                    skip_group_check=True,
                                )
                            for gi in range(4):
                                nc.tensor.matmul(
                                    Z[:, gi, :], wtile[:, d * 4 + gi, :], rhs,
                                    start=(zx is None), stop=(zx is None or gi == 3),
                                    skip_group_check=(zx is not None),
                                )
                            sig = epools[d].tile([H, 3, BC], F32, tag="sig")
                            nc.scalar.activation(sig, Z[:, 0:3, :], AF.Sigmoid)
                            nc.scalar.activation(cg[:, d, 1, :], Z[:, 3, :],
                                                 AF.Tanh)
                            m = epools[d].tile([H, 2, BC], F32, tag="m")
                            nc.vector.tensor_mul(m, sig[:, 1:3, :], cg[:, d, :, :])
                            nc.vector.tensor_add(cg[:, d, 0, :], m[:, 0, :],
                                                 m[:, 1, :])
                            ct = epools[d].tile([H, BC], F32, tag="ct")
                            nc.scalar.activation(ct, cg[:, d, 0, :], AF.Tanh)
                            nc.vector.tensor_mul(
                                S[0:H, d, wr * BC:(wr + 1) * BC],
                                sig[:, 0, :], ct,
                            )

            # ---- layer 0 ----
            with nc.named_scope("L0"):
                lstm_steps(S0, cg0, sw0, 128, None)

            # ---- Zx1 = Wih1 @ concat(hf0, hb0)  (streamed) ----
            NCH = TB // 512 if TB % 512 == 0 else TB // 512 + 1
            with tc.tile_pool(name="zx", bufs=1) as zxpool:
                Zx1 = zxpool.tile([H, 2, 4, TB], BF16)
                with nc.named_scope("ZX1"):
                    with (
                        tc.tile_pool(name="zxp", bufs=4, space="PSUM") as zxp,
                    ):
                        for dgi in range(8):
                            d, gi = divmod(dgi, 4)
                            for n in range(NCH):
                                s = n * 512
                                w = min(512, TB - s)
                                P = zxp.tile([H, 512], F32, tag="p")
                                nc.tensor.matmul(
                                    P[:, :w], sw1x[:, dgi * 2 + 0, :],
                                    S0[0:H, 0, BC + s:BC + s + w],
                                    start=True, stop=False)
                                nc.tensor.matmul(
                                    P[:, :w], sw1x[:, dgi * 2 + 1, :],
                                    S0[0:H, 1, s:s + w],
                                    start=False, stop=True)
                                dst = Zx1[:, d, gi, s:s + w]
                                if n % 2 == 0:
                                    nc.vector.tensor_copy(dst, P[:, :w])
                                else:
                                    nc.scalar.copy(dst, P[:, :w])

                # ---- layer 1 ----
                with nc.named_scope("L1"):
                    lstm_steps(S1, cg1, sw1, 128, Zx1)
                if dump:
                    nc.gpsimd.dma_start(out=dZx1[:, :, :, :], in_=Zx1)

            # ---- f head ----
            with nc.named_scope("F"):
                with (
                    tc.tile_pool(name="fp", bufs=4, space="PSUM") as fp,
                    tc.tile_pool(name="fs", bufs=4) as fs,
                ):
                    for n in range(NCH):
                        s = n * 512
                        w = min(512, TB - s)
                        P = fp.tile([LOUT, 512], F32, tag="p")
                        nc.tensor.matmul(P[:, :w], sfw[:, 0, :],
                                         S1[0:H, 0, BC + s:BC + s + w],
                                         start=True, stop=False)
                        nc.tensor.matmul(P[:, :w], sfw[:, 1, :],
                                         S1[0:H, 1, s:s + w],
                                         start=False, stop=True)
                        st = fs.tile([LOUT, 512], F32, tag="s")
                        nc.vector.tensor_scalar_add(st[:, :w], P[:, :w], sfb)
                        nc.gpsimd.dma_start(out=fo[:, s:s + w], in_=st[:, :w])

            # ---- g head (adjacent-pair concat via shifted slices) ----
            GB = (T - 1) * BC
            GCH = GB // 512 + (1 if GB % 512 else 0)
            with nc.named_scope("G"):
                with (
                    tc.tile_pool(name="gp", bufs=4, space="PSUM") as gp,
                    tc.tile_pool(name="gs", bufs=4) as gs,
                ):
                    for mi in range(4):
                        for n in range(GCH):
                            s = n * 512
                            w = min(512, GB - s)
                            P = gp.tile([H, 512], F32, tag="p")
                            rhs_k = (
                                S1[0:H, 0, BC + s:BC + s + w],    # hf1_t
                                S1[0:H, 1, s:s + w],              # hb1_t
                                S1[0:H, 0, 2 * BC + s:2 * BC + s + w],  # hf1_{t+1}
                                S1[0:H, 1, BC + s:BC + s + w],    # hb1_{t+1}
                            )
                            for k in range(4):
                                nc.tensor.matmul(
                                    P[:, :w], sgw[:, mi * 4 + k, :], rhs_k[k],
                                    start=(k == 0), stop=(k == 3))
                            st = gs.tile([H, 512], F32, tag="s")
                            nc.vector.tensor_scalar_add(st[:, :w], P[:, :w],
                                                        sgb[:, mi:mi + 1])
                            nc.gpsimd.dma_start(
                                out=go[mi * H:(mi + 1) * H, s:s + w],
                                in_=st[:, :w])

            if dump:
                nc.gpsimd.dma_start(out=dS0[:, :, :], in_=S0)
                nc.gpsimd.dma_start(out=dS1[:, :, :], in_=S1)

    nc.finalize()
    return nc


def _pack_weights(inputs, T):
    """Host-side packing of replicated weights into device layouts."""
    f32 = np.float32
    bf16 = ml_dtypes.bfloat16
    emb = np.asarray(inputs["emb"], f32)

    def gates(W):
        return [np.asarray(W, f32)[r] for r in _GATE_ROWS]

    out = {}
    # layer 0: [Whh | Wih@emb.T | b].T per (dir, gate)
    w0 = np.zeros((128, 8, H), f32)
    for d, sfx in enumerate(("f", "b")):
        Wih = np.asarray(inputs[f"Wih0{sfx}"], f32)
        Whh = np.asarray(inputs[f"Whh0{sfx}"], f32)
        bb = np.asarray(inputs[f"b0{sfx}"], f32)
        Wihp = Wih @ emb.T  # [4H, V]
        for gi, r in enumerate(_GATE_ROWS):
            w0[0:H, d * 4 + gi, :] = Whh[r].T
            w0[H:H + V, d * 4 + gi, :] = Wihp[r].T
            w0[H + V, d * 4 + gi, :] = bb[r]
    out["w0"] = w0.astype(bf16)

    # layer 1 recurrent: [Whh1 | b1].T
    w1 = np.zeros((128, 8, H), f32)
    w1x = np.zeros((H, 16, H), f32)
    for d, sfx in enumerate(("f", "b")):
        Wih = np.asarray(inputs[f"Wih1{sfx}"], f32)
        Whh = np.asarray(inputs[f"Whh1{sfx}"], f32)
        bb = np.asarray(inputs[f"b1{sfx}"], f32)
        for gi, r in enumerate(_GATE_ROWS):
            w1[0:H, d * 4 + gi, :] = Whh[r].T
            w1[H, d * 4 + gi, :] = bb[r]
            w1x[:, (d * 4 + gi) * 2 + 0, :] = Wih[r, 0:H].T
            w1x[:, (d * 4 + gi) * 2 + 1, :] = Wih[r, H:2 * H].T
    out["w1"] = w1.astype(bf16)
    out["w1x"] = w1x.astype(bf16)
    out["ident"] = np.eye(H, dtype=f32).astype(bf16)

    f_w = np.asarray(inputs["f_w"], f32)  # [20, 200]
    fw = np.zeros((H, 2, LOUT), f32)
    fw[:, 0, :] = f_w[:, 0:H].T
    fw[:, 1, :] = f_w[:, H:2 * H].T
    out["fw"] = fw.astype(bf16)
    out["fbias"] = np.asarray(inputs["f_b"], f32).reshape(LOUT, 1)

    g_w = np.asarray(inputs["g_w"], f32)  # [400, 400]
    gw = np.zeros((H, 16, H), f32)
    gb = np.zeros((H, 4), f32)
    for mi in range(4):
        for k in range(4):
            gw[:, mi * 4 + k, :] = g_w[mi * H:(mi + 1) * H, k * H:(k + 1) * H].T
        gb[:, mi] = np.asarray(inputs["g_b"], f32)[mi * H:(mi + 1) * H]
    out["gw"] = gw.astype(bf16)
    out["gbias"] = gb
    return out


def _per_core_inputs(inputs, T, core):
    f32 = np.float32
    bf16 = ml_dtypes.bfloat16
    sl = slice(core * BC, (core + 1) * BC)
    tok = np.asarray(inputs["tokens"])[sl, :T]  # [BC, T]
    TB = T * BC
    # oh[v, t*BC + b] = (tok[b, t] == v); last row = constant 1.0 (bias)
    oh = np.zeros((28, TB), np.float32)
    oh[:V] = (np.arange(V)[:, None, None] == tok.T[None, :, :]).reshape(V, TB)
    oh[V] = 1.0
    h0 = np.asarray(inputs["h0"], f32)[:, sl, :]  # [4, BC, H]
    c0 = np.asarray(inputs["c0"], f32)[:, sl, :]
    return {
        "oh": oh.astype(bf16),
        "ones1": np.vstack([np.ones((1, 2 * (T + 1) * BC), f32),
                            np.zeros((27, 2 * (T + 1) * BC), f32)]).astype(bf16),
        "h0sb": np.ascontiguousarray(h0.transpose(2, 0, 1)).astype(bf16),
        "c0s": np.ascontiguousarray(c0.transpose(2, 0, 1)).astype(f32),
    }


_NC_CACHE = {}


def _get_nc(T, dump=False):
    if (T, dump) not in _NC_CACHE:
        _NC_CACHE[(T, dump)] = _build_nc(T, dump)
    return _NC_CACHE[(T, dump)]


def run(inputs, T=T_FULL, trace=False, dump=False):
    nc = _get_nc(T, dump)
    wmaps = _pack_weights(inputs, T)
    in_maps = []
    for core in range(NCORES):
        m = dict(wmaps)
        m.update(_per_core_inputs(inputs, T, core))
        in_maps.append(m)
    res = run_bass_kernel_spmd(nc, in_maps, core_ids=list(range(NCORES)),
                               trace=trace)
    TBc = T * BC
    f_parts, g_parts = [], []
    for core in range(NCORES):
        r = res.results[core]
        f = r["fo"].reshape(LOUT, T, BC).transpose(2, 1, 0)      # [BC, T, 20]
        g = r["go"].reshape(4 * H, T - 1, BC).transpose(2, 1, 0)  # [BC, T-1, 400]
        f_parts.append(f)
        g_parts.append(g)
    f_out = np.concatenate(f_parts, 0).reshape(B * T, LOUT)
    g_out = np.concatenate(g_parts, 0).reshape(B * (T - 1), 4 * H)
    return (f_out, g_out), res


def kernel(**inputs):
    (f_out, g_out), _ = run(inputs, T=T_FULL, trace=False)
    return (f_out.astype(np.float32), g_out.astype(np.float32))


# revision 19
# speedup vs baseline: 1.0418x; 1.0418x over previous
"""Trainium2 Bass kernel for nn_Net_31722628448620.

2-layer bidirectional LSTM (B=64, T=1024, E=32, H=100, V=21) + two linear
heads (f: [B*T,20], g on adjacent-pair concat: [B*(T-1),400]).

Sharding: data-parallel over batch, 8 sequences per NeuronCore, weights
replicated. All real FLOPs run on device; the host only re-packs layouts.

Key device-side structure (per core, BC=8 sequences):
  - Layer 0 folds the embedding + input projection into the recurrent
    matmul: x_t = emb[token] lives in a V=21-dim space, so with
    Wihp = Wih @ emb.T the per-step input term is Wihp @ onehot(token).
    Contraction K = H(100) + V(21) + 1(bias) = 122 <= 128, so each gate
    is ONE matmul per step against an augmented state tile
    [h_{t-1}; onehot(token_t); 1].
  - The augmented state tile S doubles as the layer-output storage:
    step t's DVE h-write lands exactly where step t+1's matmul reads.
  - Layer 1's input transform Zx1 = Wih1 @ concat(hf0, hb0) is a big
    streaming matmul between the layer phases; during the layer-1
    recurrence it is injected into PSUM with an identity matmul
    (accumulating on top of the 4 gate matmuls), so the per-step
    elementwise work stays at 3 ACT + 3 DVE instructions per direction.
  - The pairwise concat for g is free: g's four K-chunks are just
    time-shifted slices of the layer-1 state storage.

Outputs are produced transposed ([20, T*8] / [400, (T-1)*8] per core);
the host assembles/transposes into the reference layout.
"""

import sys
import os

for _p in ("/opt/trn_rl_repo", "/root/.axon_site/_ro/trn_rl_repo"):
    if os.path.isdir(_p) and _p not in sys.path:
        sys.path.append(_p)

import numpy as np
import ml_dtypes

import concourse.bass as bass
import concourse.tile as tile
from concourse import mybir, bacc
from concourse.bass_utils import run_bass_kernel_spmd

BF16 = mybir.dt.bfloat16
F32 = mybir.dt.float32
AF = mybir.ActivationFunctionType

B, T_FULL, E, H, V = 64, 1024, 32, 100, 21
LOUT = 20
NCORES = 8
BC = B // NCORES  # sequences per core (8)

# gate order used on-device: [o, f, i, g]; torch weight row order is i,f,g,o
_GATE_ROWS = [slice(3 * H, 4 * H), slice(H, 2 * H), slice(0, H), slice(2 * H, 3 * H)]


def _build_nc(T: int, dump: bool = False) -> bass.Bass:
    """Build the full Bass program for one core (SPMD across 8)."""
    TB = T * BC            # time-major col count of one direction's states
    NBLK = (T + 1) * BC    # + one extra block for the initial state

    nc = bacc.Bacc("TRN2", target_bir_lowering=False, debug=False,
                   num_devices=NCORES)
    dS0 = dS1 = dZx1 = None
    if dump:
        dS0 = nc.dram_tensor("dS0", [128, 2, NBLK], BF16, kind="ExternalOutput")
        dS1 = nc.dram_tensor("dS1", [128, 2, NBLK], BF16, kind="ExternalOutput")
        dZx1 = nc.dram_tensor("dZx1", [H, 2, 4, TB], BF16, kind="ExternalOutput")

    # ---- I/O ----
    # oh carries V onehot rows + a constant-1.0 row (compute engines cannot
    # address a 1-partition slice at partition 121, so the bias row comes
    # in via DMA with the onehots)
    oh = nc.dram_tensor("oh", [28, TB], BF16, kind="ExternalInput")
    ones1 = nc.dram_tensor("ones1", [28, 2 * NBLK], BF16, kind="ExternalInput")
    w0 = nc.dram_tensor("w0", [128, 8, H], BF16, kind="ExternalInput")
    w1 = nc.dram_tensor("w1", [128, 8, H], BF16, kind="ExternalInput")
    w1x = nc.dram_tensor("w1x", [H, 16, H], BF16, kind="ExternalInput")
    ident = nc.dram_tensor("ident", [H, H], BF16, kind="ExternalInput")
    fw = nc.dram_tensor("fw", [H, 2, LOUT], BF16, kind="ExternalInput")
    fbias = nc.dram_tensor("fbias", [LOUT, 1], F32, kind="ExternalInput")
    gw = nc.dram_tensor("gw", [H, 16, H], BF16, kind="ExternalInput")
    gbias = nc.dram_tensor("gbias", [H, 4], F32, kind="ExternalInput")
    h0sb = nc.dram_tensor("h0sb", [H, 4, BC], BF16, kind="ExternalInput")
    c0s = nc.dram_tensor("c0s", [H, 4, BC], F32, kind="ExternalInput")

    fo = nc.dram_tensor("fo", [LOUT, TB], F32, kind="ExternalOutput")
    go = nc.dram_tensor("go", [4 * H, (T - 1) * BC], F32, kind="ExternalOutput")

    with tile.TileContext(nc) as tc:
        with (
            tc.tile_pool(name="singles", bufs=1) as singles,
            tc.tile_pool(name="stateA", bufs=1) as stateA,
            tc.tile_pool(name="stateB", bufs=1) as stateB,
        ):
            # ---- load weights / constants ----
            sw0 = singles.tile([128, 8, H], BF16)
            nc.gpsimd.dma_start(out=sw0, in_=w0[:, :, :])
            sw1 = singles.tile([128, 8, H], BF16)
            nc.gpsimd.dma_start(out=sw1, in_=w1[:, :, :])
            sw1x = singles.tile([H, 16, H], BF16)
            nc.gpsimd.dma_start(out=sw1x, in_=w1x[:, :, :])
            sid = singles.tile([H, H], BF16)
            nc.gpsimd.dma_start(out=sid, in_=ident[:, :])
            sfw = singles.tile([H, 2, LOUT], BF16)
            nc.gpsimd.dma_start(out=sfw, in_=fw[:, :, :])
            sfb = singles.tile([LOUT, 1], F32)
            nc.gpsimd.dma_start(out=sfb, in_=fbias[:, :])
            sgw = singles.tile([H, 16, H], BF16)
            nc.gpsimd.dma_start(out=sgw, in_=gw[:, :, :])
            sgb = singles.tile([H, 4], F32)
            nc.gpsimd.dma_start(out=sgb, in_=gbias[:, :])

            # ---- state storage ----
            # S0: [128 partitions, dir, (T+1)*BC]
            #   rows 0:100   h states of layer-0
            #   rows 100:121 onehot(token) (per direction, shifted)
            #   row 121      constant 1.0 (bias row for the aug matmul)
            S0 = stateA.tile([128, 2, NBLK], BF16)
            # forward: block t = [h_{t-1}; onehot(tok_t); 1]; h_t -> block t+1
            nc.gpsimd.dma_start(out=S0[H:128, 0, 0:TB], in_=oh[:, :])
            # backward: block j = [h_b(j); onehot(tok_{j-1}); 1] (j=1..T);
            # h_b(t) -> block t; initial state at block T
            nc.gpsimd.dma_start(out=S0[H:128, 1, BC:BC + TB], in_=oh[:, :])
            nc.gpsimd.dma_start(out=S0[0:H, 0, 0:BC], in_=h0sb[:, 0, :])
            nc.gpsimd.dma_start(out=S0[0:H, 1, T * BC:(T + 1) * BC],
                                in_=h0sb[:, 1, :])

            # S1: layer-1 h states; row 100 = 1.0 (bias row)
            S1 = stateB.tile([128, 2, NBLK], BF16)
            nc.gpsimd.dma_start(out=S1[H:128, :, :], in_=ones1[:, :])
            nc.gpsimd.dma_start(out=S1[0:H, 0, 0:BC], in_=h0sb[:, 2, :])
            nc.gpsimd.dma_start(out=S1[0:H, 1, T * BC:(T + 1) * BC],
                                in_=h0sb[:, 3, :])

            # persistent [c | ghat] per (layer, dir)
            cg0 = singles.tile([H, 2, 2, BC], F32)
            nc.gpsimd.dma_start(out=cg0[:, 0, 0, :], in_=c0s[:, 0, :])
            nc.gpsimd.dma_start(out=cg0[:, 1, 0, :], in_=c0s[:, 1, :])
            cg1 = singles.tile([H, 2, 2, BC], F32)
            nc.gpsimd.dma_start(out=cg1[:, 0, 0, :], in_=c0s[:, 2, :])
            nc.gpsimd.dma_start(out=cg1[:, 1, 0, :], in_=c0s[:, 3, :])

            # ---- recurrence helper ----
            def lstm_steps(S, cg, wtile, krows, zx):
                """Emit one bidirectional layer's T steps."""
                with (
                    tc.tile_pool(name="zp0", bufs=4, space="PSUM") as zp0,
                    tc.tile_pool(name="zp1", bufs=4, space="PSUM") as zp1,
                    tc.tile_pool(name="ew0", bufs=3) as ew0,
                    tc.tile_pool(name="ew1", bufs=3) as ew1,
                ):
                    zpools = (zp0, zp1)
                    epools = (ew0, ew1)
                    for t in range(T):
                        for d in (0, 1):
                            tseq = t if d == 0 else T - 1 - t
                            rd = tseq if d == 0 else tseq + 1
                            wr = tseq + 1 if d == 0 else tseq
                            Z = zpools[d].tile([H, 4, BC], F32, tag="z")
                            rhs = S[0:krows, d, rd * BC:(rd + 1) * BC]
                            if zx is not None:
                                # inject Zx1 first (identity matmul), gates
                                # then accumulate on top of it
                                nc.tensor.matmul(
                                    Z[:, :, :], sid,
                                    zx[:, d, :, tseq * BC:(tseq + 1) * BC],
                                    start=True, stop=False,
                                    skip_group_check=True,
                                )
                            for gi in range(4):
                                nc.tensor.matmul(
                                    Z[:, gi, :], wtile[:, d * 4 + gi, :], rhs,
                                    start=(zx is None), stop=(zx is None or gi == 3),
                                    skip_group_check=(zx is not None),
                                )
                            # gate order [o,f,i,g]; g's weights are pre-scaled
                            # by 2 so sigmoid(2x) = (tanh(x)+1)/2 covers all
                            # four gates in ONE activation
                            sig = epools[d].tile([H, 4, BC], F32, tag="sig")
                            nc.scalar.activation(sig, Z[:, :, :], AF.Sigmoid)
                            nc.vector.tensor_scalar(
                                out=cg[:, d, 1, :], in0=sig[:, 3, :],
                                scalar1=2.0, scalar2=1.0,
                                op0=mybir.AluOpType.mult,
                                op1=mybir.AluOpType.subtract)
                            m = epools[d].tile([H, 2, BC], F32, tag="m")
                            nc.vector.tensor_mul(m, sig[:, 1:3, :], cg[:, d, :, :])
                            nc.vector.tensor_add(cg[:, d, 0, :], m[:, 0, :],
                                                 m[:, 1, :])
                            ct = epools[d].tile([H, BC], F32, tag="ct")
                            nc.scalar.activation(ct, cg[:, d, 0, :], AF.Tanh)
                            nc.vector.tensor_mul(
                                S[0:H, d, wr * BC:(wr + 1) * BC],
                                sig[:, 0, :], ct,
                            )

            # ---- layer 0 ----
            with nc.named_scope("L0"):
                lstm_steps(S0, cg0, sw0, 128, None)

            # ---- Zx1 = Wih1 @ concat(hf0, hb0)  (streamed) ----
            NCH = TB // 512 if TB % 512 == 0 else TB // 512 + 1
            with tc.tile_pool(name="zx", bufs=1) as zxpool:
                Zx1 = zxpool.tile([H, 2, 4, TB], BF16)
                with nc.named_scope("ZX1"):
                    with (
                        tc.tile_pool(name="zxp", bufs=4, space="PSUM") as zxp,
                    ):
                        for dgi in range(8):
                            d, gi = divmod(dgi, 4)
                            for n in range(NCH):
                                s = n * 512
                                w = min(512, TB - s)
                                P = zxp.tile([H, 512], F32, tag="p")
                                nc.tensor.matmul(
                                    P[:, :w], sw1x[:, dgi * 2 + 0, :],
                                    S0[0:H, 0, BC + s:BC + s + w],
                                    start=True, stop=False)
                                nc.tensor.matmul(
                                    P[:, :w], sw1x[:, dgi * 2 + 1, :],
                                    S0[0:H, 1, s:s + w],
                                    start=False, stop=True)
                                dst = Zx1[:, d, gi, s:s + w]
                                if n % 2 == 0:
                                    nc.vector.tensor_copy(dst, P[:, :w])
                                else:
                                    nc.scalar.copy(dst, P[:, :w])

                # ---- layer 1 ----
                with nc.named_scope("L1"):
                    lstm_steps(S1, cg1, sw1, 128, Zx1)
                if dump:
                    nc.gpsimd.dma_start(out=dZx1[:, :, :, :], in_=Zx1)

            # ---- f head ----
            with nc.named_scope("F"):
                with (
                    tc.tile_pool(name="fp", bufs=4, space="PSUM") as fp,
                    tc.tile_pool(name="fs", bufs=4) as fs,
                ):
                    for n in range(NCH):
                        s = n * 512
                        w = min(512, TB - s)
                        P = fp.tile([LOUT, 512], F32, tag="p")
                        nc.tensor.matmul(P[:, :w], sfw[:, 0, :],
                                         S1[0:H, 0, BC + s:BC + s + w],
                                         start=True, stop=False)
                        nc.tensor.matmul(P[:, :w], sfw[:, 1, :],
                                         S1[0:H, 1, s:s + w],
                                         start=False, stop=True)
                        st = fs.tile([LOUT, 512], F32, tag="s")
                        nc.vector.tensor_scalar_add(st[:, :w], P[:, :w], sfb)
                        nc.gpsimd.dma_start(out=fo[:, s:s + w], in_=st[:, :w])

            # ---- g head (adjacent-pair concat via shifted slices) ----
            GB = (T - 1) * BC
            GCH = GB // 512 + (1 if GB % 512 else 0)
            with nc.named_scope("G"):
                with (
                    tc.tile_pool(name="gp", bufs=4, space="PSUM") as gp,
                    tc.tile_pool(name="gs", bufs=4) as gs,
                ):
                    for mi in range(4):
                        for n in range(GCH):
                            s = n * 512
                            w = min(512, GB - s)
                            P = gp.tile([H, 512], F32, tag="p")
                            rhs_k = (
                                S1[0:H, 0, BC + s:BC + s + w],    # hf1_t
                                S1[0:H, 1, s:s + w],              # hb1_t
                                S1[0:H, 0, 2 * BC + s:2 * BC + s + w],  # hf1_{t+1}
                                S1[0:H, 1, BC + s:BC + s + w],    # hb1_{t+1}
                            )
                            for k in range(4):
                                nc.tensor.matmul(
                                    P[:, :w], sgw[:, mi * 4 + k, :], rhs_k[k],
                                    start=(k == 0), stop=(k == 3))
                            st = gs.tile([H, 512], F32, tag="s")
                            nc.vector.tensor_scalar_add(st[:, :w], P[:, :w],
                                                        sgb[:, mi:mi + 1])
                            nc.gpsimd.dma_start(
                                out=go[mi * H:(mi + 1) * H, s:s + w],
                                in_=st[:, :w])

            if dump:
                nc.gpsimd.dma_start(out=dS0[:, :, :], in_=S0)
                nc.gpsimd.dma_start(out=dS1[:, :, :], in_=S1)

    nc.finalize()
    return nc


def _pack_weights(inputs, T):
    """Host-side packing of replicated weights into device layouts."""
    f32 = np.float32
    bf16 = ml_dtypes.bfloat16
    emb = np.asarray(inputs["emb"], f32)

    def gates(W):
        return [np.asarray(W, f32)[r] for r in _GATE_ROWS]

    out = {}
    # layer 0: [Whh | Wih@emb.T | b].T per (dir, gate)
    w0 = np.zeros((128, 8, H), f32)
    for d, sfx in enumerate(("f", "b")):
        Wih = np.asarray(inputs[f"Wih0{sfx}"], f32)
        Whh = np.asarray(inputs[f"Whh0{sfx}"], f32)
        bb = np.asarray(inputs[f"b0{sfx}"], f32)
        Wihp = Wih @ emb.T  # [4H, V]
        for gi, r in enumerate(_GATE_ROWS):
            sc = 2.0 if gi == 3 else 1.0  # g-gate pre-scaled for tanh-via-sigmoid
            w0[0:H, d * 4 + gi, :] = sc * Whh[r].T
            w0[H:H + V, d * 4 + gi, :] = sc * Wihp[r].T
            w0[H + V, d * 4 + gi, :] = sc * bb[r]
    out["w0"] = w0.astype(bf16)

    # layer 1 recurrent: [Whh1 | b1].T
    w1 = np.zeros((128, 8, H), f32)
    w1x = np.zeros((H, 16, H), f32)
    for d, sfx in enumerate(("f", "b")):
        Wih = np.asarray(inputs[f"Wih1{sfx}"], f32)
        Whh = np.asarray(inputs[f"Whh1{sfx}"], f32)
        bb = np.asarray(inputs[f"b1{sfx}"], f32)
        for gi, r in enumerate(_GATE_ROWS):
            sc = 2.0 if gi == 3 else 1.0
            w1[0:H, d * 4 + gi, :] = sc * Whh[r].T
            w1[H, d * 4 + gi, :] = sc * bb[r]
            w1x[:, (d * 4 + gi) * 2 + 0, :] = sc * Wih[r, 0:H].T
            w1x[:, (d * 4 + gi) * 2 + 1, :] = sc * Wih[r, H:2 * H].T
    out["w1"] = w1.astype(bf16)
    out["w1x"] = w1x.astype(bf16)
    out["ident"] = np.eye(H, dtype=f32).astype(bf16)

    f_w = np.asarray(inputs["f_w"], f32)  # [20, 200]
    fw = np.zeros((H, 2, LOUT), f32)
    fw[:, 0, :] = f_w[:, 0:H].T
    fw[:, 1, :] = f_w[:, H:2 * H].T
    out["fw"] = fw.astype(bf16)
    out["fbias"] = np.asarray(inputs["f_b"], f32).reshape(LOUT, 1)

    g_w = np.asarray(inputs["g_w"], f32)  # [400, 400]
    gw = np.zeros((H, 16, H), f32)
    gb = np.zeros((H, 4), f32)
    for mi in range(4):
        for k in range(4):
            gw[:, mi * 4 + k, :] = g_w[mi * H:(mi + 1) * H, k * H:(k + 1) * H].T
        gb[:, mi] = np.asarray(inputs["g_b"], f32)[mi * H:(mi + 1) * H]
    out["gw"] = gw.astype(bf16)
    out["gbias"] = gb
    return out


def _per_core_inputs(inputs, T, core):
    f32 = np.float32
    bf16 = ml_dtypes.bfloat16
    sl = slice(core * BC, (core + 1) * BC)
    tok = np.asarray(inputs["tokens"])[sl, :T]  # [BC, T]
    TB = T * BC
    # oh[v, t*BC + b] = (tok[b, t] == v); last row = constant 1.0 (bias)
    oh = np.zeros((28, TB), np.float32)
    oh[:V] = (np.arange(V)[:, None, None] == tok.T[None, :, :]).reshape(V, TB)
    oh[V] = 1.0
    h0 = np.asarray(inputs["h0"], f32)[:, sl, :]  # [4, BC, H]
    c0 = np.asarray(inputs["c0"], f32)[:, sl, :]
    return {
        "oh": oh.astype(bf16),
        "ones1": np.vstack([np.ones((1, 2 * (T + 1) * BC), f32),
                            np.zeros((27, 2 * (T + 1) * BC), f32)]).astype(bf16),
        "h0sb": np.ascontiguousarray(h0.transpose(2, 0, 1)).astype(bf16),
        "c0s": np.ascontiguousarray(c0.transpose(2, 0, 1)).astype(f32),
    }


_NC_CACHE = {}


def _get_nc(T, dump=False):
    if (T, dump) not in _NC_CACHE:
        _NC_CACHE[(T, dump)] = _build_nc(T, dump)
    return _NC_CACHE[(T, dump)]


def run(inputs, T=T_FULL, trace=False, dump=False):
    nc = _get_nc(T, dump)
    wmaps = _pack_weights(inputs, T)
    in_maps = []
    for core in range(NCORES):
        m = dict(wmaps)
        m.update(_per_core_inputs(inputs, T, core))
        in_maps.append(m)
    res = run_bass_kernel_spmd(nc, in_maps, core_ids=list(range(NCORES)),
                               trace=trace)
    TBc = T * BC
    f_parts, g_parts = [], []
    for core in range(NCORES):
        r = res.results[core]
        f = r["fo"].reshape(LOUT, T, BC).transpose(2, 1, 0)      # [BC, T, 20]
        g = r["go"].reshape(4 * H, T - 1, BC).transpose(2, 1, 0)  # [BC, T-1, 400]
        f_parts.append(f)
        g_parts.append(g)
    f_out = np.concatenate(f_parts, 0).reshape(B * T, LOUT)
    g_out = np.concatenate(g_parts, 0).reshape(B * (T - 1), 4 * H)
    return (f_out, g_out), res


def kernel(**inputs):
    (f_out, g_out), _ = run(inputs, T=T_FULL, trace=False)
    return (f_out.astype(np.float32), g_out.astype(np.float32))
